# revision 41
# baseline (speedup 1.0000x reference)
"""Trainium2 Bass kernel for nn_DeformableConvLSTMCell_33895881900284.

Full (unsharded) inputs in, full outputs out. Data-parallel over batch across
8 NeuronCores (8 batches per core), conv weights / gate params replicated.

Math per the reference:
  outI  = conv3x3_same(inputs, wconvInput)
  g     = tanh(outI + conv3x3_same(hidden_prev, wconvHidden) + gateBias)
  gapI  = mean_hw(outI);  gapH = mean_hw(hidden_prev)          # [B, D]
  i/f/o = sigmoid(wx*gapI + wh*gapH + bias)                    # [B, D]
  tiled gate: value used at (b, h, w, c) is gate[(28*b + h) % 64, c]
  state  = f*state_prev + i*g;  hidden = o*tanh(state)

v2 design (vs the f32r baseline):
  * Host pre-transposes inputs/hidden/state to channel-major [BL, cc, 128,
    784] (layout-only, like the baseline's host-side output untranspose), so
    the kernel does no PE transposes and no DVE layout copies.
  * The whole conv path runs in bf16: weights/inputs/gateBias are cast on
    the host, conv = 36 shifted bf16 matmuls + a bias identity-matmul per
    392-pixel window accumulating in fp32 PSUM. bf16 enables the PE's Fast
    Weight Load path (fp32r disables it), roughly halving per-matmul cost.
  * gapI never touches the conv output: by linearity 784*gapI is a
    combination of 9 masked pixel sums of the raw input with host-folded
    A-matrices (sums of conv taps). The masked sums (full/edge/corner) are
    free-dim DVE reductions over the staged channel-major tiles, so the
    AllGather of gate drivers fires ~45us in, long before the first
    elementwise consumer.
  * The (28*b+h)%64 gate scrambling makes gates cross-batch: cores AllGather
    their local GAP columns and a per-core index-array input drives an
    indirect-DMA gather of exactly the gate rows this core needs (the SPMD
    program stays identical across cores; only input data differs).
"""
import numpy as np
import ml_dtypes

import bass_rust
import concourse.bass as bass
import concourse.mybir as mybir
import concourse.tile as tile
from concourse.bass_utils import run_bass_kernel_spmd

F32 = mybir.dt.float32
BF16 = mybir.dt.bfloat16
I32 = mybir.dt.int32
AF = mybir.ActivationFunctionType
ALU = mybir.AluOpType

N_CORES = 8
B, H, W, CIN, D = 64, 28, 28, 256, 256
BL = B // N_CORES          # local batches per core
PIX = H * W                # 784
PAD = 30                   # padded row/col length
XTLEN = PAD * PAD          # 900
NW = 2                     # windows per batch
WROWS = H // NW            # 14
WN = WROWS * W             # 392
WSTR = WROWS * PAD         # 420: window length in row-stride-30 layout
WFLAT = (WROWS - 1) * PAD + W   # 418: flat contiguous conv stream length
PPIX = H * PAD             # 840: per-image length in row-stride-30 layout
NCC = CIN // 128           # 2 input-channel chunks
NDC = D // 128             # 2 output-channel chunks

# tap order t = 3*kh + kw ; dh = kh-1, dw = kw-1
TAPS = [(kh, kw) for kh in range(3) for kw in range(3)]

# ---------------------------------------------------------------------------
# walrus fixup: split semaphore waits that exceed the per-instruction budget
MAX_WAITS = 1


def _split_excess_sem_waits(nc):
    counter = [0]
    for fn in nc.m.functions:
        for bb in fn.blocks:
            insts = bb.instructions
            i = 0
            while i < len(insts):
                inst = insts[i]
                si = inst.sync_info
                if si is not None and si.on_wait and len(si.on_wait) > MAX_WAITS:
                    waits = list(si.on_wait)
                    excess = waits[:-MAX_WAITS]
                    keep = waits[-MAX_WAITS:]
                    new_insts = []
                    for j in range(0, len(excess), MAX_WAITS):
                        chunk = excess[j:j + MAX_WAITS]
                        noop = mybir.InstNoOp(
                            name=f"I-waitsplit-{counter[0]}", ins=[], outs=[])
                        counter[0] += 1
                        noop.engine = inst.engine
                        noop.sync_info = bass_rust.SyncInfo(
                            on_wait=chunk, on_update=[])
                        nc.register_instruction(noop)
                        new_insts.append(noop)
                    inst.sync_info = bass_rust.SyncInfo(
                        on_wait=keep, on_update=list(si.on_update))
                    insts[i:i] = new_insts
                    i += len(new_insts)
                i += 1
    return nc


# ---------------------------------------------------------------------------
def _r3(ap, w):
    """view flat free dim as (rows, w)"""
    return ap.rearrange("c (r w) -> c r w", w=w)


def build_nc():
    nc = bass.Bass("TRN2", target_bir_lowering=False, debug=False,
                   num_devices=N_CORES)

    dram = {}
    dram["xin"] = nc.dram_tensor("inputs_t", [BL, NCC, 128, XTLEN], BF16,
                                 kind="ExternalInput").ap()
    dram["xhid"] = nc.dram_tensor("hidden_t", [BL, NCC, 128, XTLEN], BF16,
                                  kind="ExternalInput").ap()
    dram["xsp"] = nc.dram_tensor("state_t", [BL, NCC, 128, PPIX], F32,
                                 kind="ExternalInput").ap()
    dram["w1"] = nc.dram_tensor("w1_bf", [9, NCC, 128, D], BF16,
                                kind="ExternalInput").ap()
    dram["w2"] = nc.dram_tensor("w2_bf", [9, NCC, 128, D], BF16,
                                kind="ExternalInput").ap()
    dram["afold"] = nc.dram_tensor("a_fold", [NCC, 9, 128, D], BF16,
                                   kind="ExternalInput").ap()
    dram["gb"] = nc.dram_tensor("gbias_t", [NDC, 128, PPIX], F32,
                                kind="ExternalInput").ap()
    dram["vec"] = {}
    for nm in ("wxi", "whi", "inputBias", "wxf", "whf", "forgetBias",
               "wxo", "who", "outputBias"):
        dram["vec"][nm] = nc.dram_tensor(nm, [128, NDC], F32,
                                         kind="ExternalInput").ap()
    dram["ident"] = nc.dram_tensor("identity", [128, 128], F32,
                                   kind="ExternalInput").ap()
    dram["idx"] = nc.dram_tensor("gate_idx", [4, 224, 1], I32,
                                 kind="ExternalInput").ap()
    dram["hidden"] = nc.dram_tensor("hidden", [BL, NDC, 128, PPIX], F32,
                                    kind="ExternalOutput").ap()
    dram["state"] = nc.dram_tensor("state", [BL, NDC, 128, PPIX], F32,
                                   kind="ExternalOutput").ap()
    dram["cc_in"] = nc.dram_tensor("cc_in", [32, 128], F32, kind="Internal").ap()
    dram["cc_out"] = nc.dram_tensor("cc_out", [N_CORES * 32, 128], F32,
                                    kind="Internal", addr_space="Shared").ap()

    ctx_mgr = nc.allow_low_precision("bf16 conv path")
    ctx_mgr.__enter__()
    with tile.TileContext(nc) as tc:
        _build_body(nc, tc, dram)
    ctx_mgr.__exit__(None, None, None)
    return nc


def _build_body(nc, tc, dram):
    from contextlib import ExitStack
    ctx = ExitStack()
    pool = lambda **kw: ctx.enter_context(tc.tile_pool(**kw))

    const = pool(name="const", bufs=1)
    xspp = pool(name="xspp", bufs=5)       # [128, 840] f32 state tiles
    gtp = pool(name="gtp", bufs=13)        # [128, 420] f32 tanh-conv tiles
    outb = pool(name="outb", bufs=3)       # [128, 840] f32 out tiles (per tag)
    ew = pool(name="ew", bufs=3)           # [128, 420] f32 scratch (per tag)
    gtmp = pool(name="gtmp", bufs=2)
    gath = pool(name="gath", bufs=2)
    ps_conv = pool(name="ps_conv", bufs=6, space="PSUM")
    ps_gap = pool(name="ps_gap", bufs=1, space="PSUM")
    ps_tr = pool(name="ps_tr", bufs=1, space="PSUM")

    # ---- constants; weights split across both DMA rings so conv(0) can
    # start ~8us in: w1 leads the sync ring, w2 follows gbias on scalar ----
    wc = [const.tile([128, 9 * NCC * D], BF16, tag=f"wc{conv}",
                     name=f"wc{conv}") for conv in range(2)]

    def wblk(conv, t, cc):
        off = (t * NCC + cc) * D
        return wc[conv][:, off:off + D]

    # dram [9,NCC,128,D] -> SBUF [128, (9,NCC,D)] in one strided DMA
    nc.sync.dma_start(
        wc[0][:].rearrange("p (t c n) -> p t c n", t=9, c=NCC),
        dram["w1"][:].rearrange("t c p n -> p t c n"))

    gbias = [const.tile([128, PPIX], F32, tag=f"gbias{dc}", name=f"gbias{dc}")
             for dc in range(NDC)]
    for dc in range(NDC):
        nc.scalar.dma_start(gbias[dc][:], dram["gb"][dc])

    nc.scalar.dma_start(
        wc[1][:].rearrange("p (t c n) -> p t c n", t=9, c=NCC),
        dram["w2"][:].rearrange("t c p n -> p t c n"))

    afold = const.tile([128, NCC * 9 * D], BF16, tag="afold")

    def ablk(cc, g):
        off = (cc * 9 + g) * D
        return afold[:, off:off + D]

    nc.scalar.dma_start(
        afold[:].rearrange("p (c g n) -> p c g n", c=NCC, g=9),
        dram["afold"][:].rearrange("c g p n -> p c g n"))

    idx_sb = []
    for g4 in range(4):
        halves = []
        for hf in range(2):
            t = const.tile([112, 1], I32, tag=f"idx{g4}_{hf}")
            nc.scalar.dma_start(t[:], dram["idx"][g4, hf * 112:(hf + 1) * 112, :])
            halves.append(t)
        idx_sb.append(halves)

    vecs = {}
    for nm in dram["vec"]:
        t = const.tile([128, NDC], F32, tag=f"vec_{nm}")
        nc.scalar.dma_start(t[:], dram["vec"][nm][:])
        vecs[nm] = t

    ident = const.tile([128, 128], F32, tag="ident")
    nc.scalar.dma_start(ident[:], dram["ident"][:])

    # ---- padded input tiles (host pre-pads the zero borders) ----
    xt = {}       # xt[(j, tensor, cc)] -> [128, 900] bf16
    for j in range(BL):
        for tn in ("in", "hid"):
            for cc in range(NCC):
                xt[(j, tn, cc)] = const.tile(
                    [128, XTLEN], BF16, tag=f"x{tn}{j}_{cc}",
                    name=f"x{tn}{j}_{cc}")

    # masked-sum accumulators (columns written per batch)
    rawI = [const.tile([128, 9 * BL], BF16, tag=f"rawI{cc}", name=f"rawI{cc}")
            for cc in range(NCC)]
    rawH = [const.tile([128, BL], F32, tag=f"rawH{cc}", name=f"rawH{cc}")
            for cc in range(NCC)]

    # ---- per-batch input loading (sync/SP DMA ring) ----
    def emit_load(j):
        for tn, dsrc in (("in", dram["xin"]), ("hid", dram["xhid"])):
            for cc in range(NCC):
                nc.sync.dma_start(xt[(j, tn, cc)][:], dsrc[j, cc])

    xsp_tiles = {}

    def emit_load_state(j):
        ts = []
        for cc in range(NCC):
            s = xspp.tile([128, PPIX], F32, tag="xsp", name=f"xsp{j}_{cc}")
            nc.sync.dma_start(s[:], dram["xsp"][j, cc])
            ts.append(s)
        xsp_tiles[j] = ts

    # ---- stage: masked-sum reductions (vector) over the padded tiles ----
    def emit_stage(j):
        for tn in ("in", "hid"):
            for cc in range(NCC):
                s = xt[(j, tn, cc)]
                s3 = _r3(s[:], PAD)                     # [128, 30, 30]
                if tn == "hid":
                    # gapH: full pixel sum only (borders are zero)
                    nc.vector.tensor_reduce(
                        rawH[cc][:, j:j + 1], s[:], mybir.AxisListType.X,
                        ALU.add)
                else:
                    rv = rawI[cc][:].rearrange("c (g b) -> c g b", b=BL)
                    # group order: S, Rf(row0), Rl(row27), Cf(col0),
                    # Cl(col27), K00, K0L, KL0, KLL
                    nc.vector.tensor_reduce(
                        rv[:, 0, j:j + 1], s[:], mybir.AxisListType.X, ALU.add)
                    nc.vector.tensor_reduce(
                        rv[:, 1, j:j + 1], s[:, PAD + 1:PAD + 1 + W],
                        mybir.AxisListType.X, ALU.add)
                    nc.vector.tensor_reduce(
                        rv[:, 2, j:j + 1], s[:, 28 * PAD + 1:28 * PAD + 1 + W],
                        mybir.AxisListType.X, ALU.add)
                    nc.vector.tensor_reduce(
                        rv[:, 3, j:j + 1], s3[:, 1:29, 1:2],
                        mybir.AxisListType.XY, ALU.add)
                    nc.vector.tensor_reduce(
                        rv[:, 4, j:j + 1], s3[:, 1:29, 28:29],
                        mybir.AxisListType.XY, ALU.add)
                    corners = s3[:, 1:29:27, 1:29:27]   # [128, 2, 2]
                    dstc = rv[:, 5:9, j:j + 1].rearrange(
                        "c (x y) o -> c x (y o)", x=2)
                    nc.vector.tensor_copy(dstc, corners)

    # ---- conv windows ----
    gt_tiles = {}

    def emit_conv(j):
        # flat-window conv: every tap streams 418 CONTIGUOUS elements of the
        # row-stride-30 padded tile; the 2 inter-row junk columns accumulate
        # harmless garbage that downstream views skip / the host strips.
        for wi in range(NW):
            for dc in range(NDC):
                h0 = 1 + wi * WROWS
                p = ps_conv.tile([128, WSTR], F32, tag="pconv", name="pconv")
                first = True
                for conv, tn in ((0, "in"), (1, "hid")):
                    for t, (kh, kw) in enumerate(TAPS):
                        dh, dwid = kh - 1, kw - 1
                        s0 = (h0 + dh) * PAD + 1 + dwid
                        for cc in range(NCC):
                            rhs = xt[(j, tn, cc)][:, s0:s0 + WFLAT]
                            last = (conv == 1 and t == 8 and cc == NCC - 1)
                            nc.tensor.matmul(
                                p[:, 0:WFLAT],
                                wblk(conv, t, cc)[:, dc * 128:(dc + 1) * 128],
                                rhs, start=first, stop=last)
                            first = False
                nc.vector.tensor_tensor(
                    out=p[:], in0=p[:],
                    in1=gbias[dc][:, wi * WSTR:(wi + 1) * WSTR], op=ALU.add)
                gt = gtp.tile([128, WSTR], F32, tag="gt", name="gt")
                nc.scalar.activation(gt[:], p[:], AF.Tanh)
                gt_tiles[(j, wi, dc)] = gt

    # ---- gap combine + AllGather ----
    def emit_combine():
        gap_ps = ps_gap.tile([8, D], F32, tag="gapI")
        for cc in range(NCC):
            rv = rawI[cc][:].rearrange("c (g b) -> c g b", b=BL)
            for g in range(9):
                nc.tensor.matmul(gap_ps[:], rv[:, g], ablk(cc, g),
                                 start=(cc == 0 and g == 0),
                                 stop=(cc == NCC - 1 and g == 8))
        gapI_sb = const.tile([8, D], F32, tag="gapI_sb")
        nc.vector.tensor_copy(gapI_sb[:], gap_ps[:])
        nc.scalar.dma_start(dram["cc_in"][0:8, :], gapI_sb[:, 0:128])
        nc.scalar.dma_start(dram["cc_in"][8:16, :], gapI_sb[:, 128:256])
        for cc in range(NCC):
            pt = ps_tr.tile([128, 128], F32, tag="ptr", name="pt_gapH")
            pt = pt[0:8, :]
            nc.tensor.transpose(pt, rawH[cc][:], ident[:])
            hs = const.tile([8, 128], F32, tag=f"gapH_sb{cc}",
                            name=f"gapHsb{cc}")
            nc.vector.tensor_copy(hs[:], pt)
            nc.scalar.dma_start(dram["cc_in"][16 + 8 * cc:24 + 8 * cc, :],
                                hs[:])
        nc.gpsimd.collective_compute(
            "AllGather", ALU.bypass, replica_groups=[list(range(N_CORES))],
            ins=[dram["cc_in"][:]], outs=[dram["cc_out"][:]])

    # ---- gather + gate tables ----
    gates = {}

    def emit_gates():
        sel = [const.tile([128, 224], F32, tag=f"sel{g4}", name=f"sel{g4}")
               for g4 in range(4)]
        for g4 in range(4):
            for hf in range(2):
                gtile = gath.tile([112, 128], F32, tag="gath", name="gath")
                nc.gpsimd.indirect_dma_start(
                    out=gtile[:], out_offset=None, in_=dram["cc_out"][:],
                    in_offset=bass.IndirectOffsetOnAxis(
                        ap=idx_sb[g4][hf][:, :1], axis=0))
                pt = ps_tr.tile([128, 128], F32, tag="ptr", name="pt_gath")
                nc.tensor.transpose(pt[:, 0:112], gtile[:],
                                    ident[0:112, 0:112])
                nc.vector.tensor_copy(sel[g4][:, hf * 112:(hf + 1) * 112],
                                      pt[:, 0:112])
        for gate, wx, wh, bi in (("i", "wxi", "whi", "inputBias"),
                                 ("f", "wxf", "whf", "forgetBias"),
                                 ("o", "wxo", "who", "outputBias")):
            per_dc = []
            for dc in range(NDC):
                t1 = gtmp.tile([128, 224], F32, tag="gm1", name="gm1")
                nc.vector.tensor_scalar_mul(t1[:], sel[dc][:],
                                            vecs[wx][:, dc:dc + 1])
                t2 = gtmp.tile([128, 224], F32, tag="gm2", name="gm2")
                nc.vector.tensor_scalar_mul(t2[:], sel[2 + dc][:],
                                            vecs[wh][:, dc:dc + 1])
                nc.vector.tensor_tensor(out=t1[:], in0=t1[:], in1=t2[:],
                                        op=ALU.add)
                gt = const.tile([128, 224], F32, tag=f"gate_{gate}{dc}",
                                name=f"gate_{gate}{dc}")
                nc.scalar.activation(gt[:], t1[:], AF.Sigmoid,
                                     bias=vecs[bi][:, dc:dc + 1])
                per_dc.append(gt)
            gates[gate] = per_dc

    # ---- elementwise + store ----
    def emit_ew(j):
        stT = [outb.tile([128, PPIX], F32, tag="stT", name=f"stT{j}_{dc}")
               for dc in range(NDC)]
        hidT = [outb.tile([128, PPIX], F32, tag="hidT", name=f"hidT{j}_{dc}")
                for dc in range(NDC)]
        for wi in range(NW):
            for dc in range(NDC):
                base = wi * WSTR
                t0 = j * H + wi * WROWS

                def gw(gate):
                    return gates[gate][dc][:, t0:t0 + WROWS].to_broadcast(
                        [128, WROWS, PAD])

                gt = gt_tiles.pop((j, wi, dc))
                sp3 = _r3(xsp_tiles[j][dc][:, base:base + WSTR], PAD)
                g3 = _r3(gt[:], PAD)
                st3 = _r3(stT[dc][:, base:base + WSTR], PAD)
                hd3 = _r3(hidT[dc][:, base:base + WSTR], PAD)
                s1 = ew.tile([128, WSTR], F32, tag="s1", name="s1")
                nc.gpsimd.tensor_tensor(out=_r3(s1[:], PAD), in0=sp3,
                                        in1=gw("f"), op=ALU.mult)
                s2 = ew.tile([128, WSTR], F32, tag="s2", name="s2")
                nc.gpsimd.tensor_tensor(out=_r3(s2[:], PAD), in0=g3,
                                        in1=gw("i"), op=ALU.mult)
                nc.vector.tensor_tensor(out=st3, in0=_r3(s1[:], PAD),
                                        in1=_r3(s2[:], PAD), op=ALU.add)
                th = ew.tile([128, WSTR], F32, tag="th", name="th")
                nc.scalar.activation(th[:], stT[dc][:, base:base + WSTR],
                                     AF.Tanh)
                nc.gpsimd.tensor_tensor(out=hd3, in0=_r3(th[:], PAD),
                                        in1=gw("o"), op=ALU.mult)
        for dname, buf in (("state", stT), ("hidden", hidT)):
            for dc in range(NDC):
                nc.scalar.dma_start(dram[dname][j, dc], buf[dc][:])

    # ================= schedule =================
    for j in range(BL):
        emit_load(j)
    for j in range(BL):
        emit_load_state(j)
    for j in range(BL):
        emit_stage(j)
    emit_conv(0)
    emit_combine()
    emit_conv(1)
    emit_conv(2)
    emit_gates()
    emit_conv(3)
    emit_ew(0)
    for j in range(4, BL):
        emit_conv(j)
        emit_ew(j - 3)
    for j in range(BL - 3, BL):
        emit_ew(j)

    ctx.close()


# ---------------------------------------------------------------------------
_NC_CACHE = None


def _get_nc():
    global _NC_CACHE
    if _NC_CACHE is None:
        nc = build_nc()
        _split_excess_sem_waits(nc)
        _NC_CACHE = nc
    return _NC_CACHE


def _gate_idx(core):
    idx = np.empty((4, 224, 1), np.int32)
    for j in range(BL):
        for hh in range(H):
            t = j * H + hh
            sel_b = (H * (BL * core + j) + hh) % B
            cp, bp = sel_b // BL, sel_b % BL
            for g in range(4):
                idx[g, t, 0] = cp * 32 + g * 8 + bp
    return idx


def _make_in_maps(inputs):
    f32 = np.float32
    bf16 = ml_dtypes.bfloat16

    w1 = np.ascontiguousarray(inputs["wconvInput"], dtype=f32)  # [3,3,CIN,D]
    w2 = np.ascontiguousarray(inputs["wconvHidden"], dtype=f32)
    # w{1,2}_bf[t, cc, 128, D]
    w1b = np.empty((9, NCC, 128, D), dtype=bf16)
    w2b = np.empty((9, NCC, 128, D), dtype=bf16)
    for wb, w in ((w1b, w1), (w2b, w2)):
        for t, (kh, kw) in enumerate(TAPS):
            for cc in range(NCC):
                wb[t, cc] = w[kh, kw, cc * 128:(cc + 1) * 128, :]

    # A-fold for gapI: 784*gapI = sum_g raw_g^T @ A_g  (group order
    # S, Rf, Rl, Cf, Cl, K00, K0L, KL0, KLL; edge groups negated)
    wt = w1.reshape(9, CIN, D)
    A = np.empty((9, CIN, D), f32)
    A[0] = wt.sum(0)
    A[1] = -(wt[6] + wt[7] + wt[8])
    A[2] = -(wt[0] + wt[1] + wt[2])
    A[3] = -(wt[2] + wt[5] + wt[8])
    A[4] = -(wt[0] + wt[3] + wt[6])
    A[5], A[6], A[7], A[8] = wt[8], wt[6], wt[2], wt[0]
    afold = np.empty((NCC, 9, 128, D), dtype=bf16)
    for cc in range(NCC):
        afold[cc] = A[:, cc * 128:(cc + 1) * 128, :]

    gb = np.ascontiguousarray(inputs["gateBias"], dtype=f32).reshape(PIX, D)
    gbias_t = np.zeros((NDC, 128, H, PAD), f32)
    gbias_t[:, :, :, 0:W] = gb.T.reshape(NDC, 128, H, W)
    gbias_t = gbias_t.reshape(NDC, 128, PPIX)

    shared = {
        "w1_bf": w1b,
        "w2_bf": w2b,
        "a_fold": afold,
        "gbias_t": gbias_t,
        "identity": np.eye(128, dtype=f32),
    }
    for nm in ("wxi", "whi", "inputBias", "wxf", "whf", "forgetBias",
               "wxo", "who", "outputBias"):
        v = np.ascontiguousarray(inputs[nm], dtype=f32).reshape(D)
        if nm.startswith("wx") or nm.startswith("wh"):
            v = v / PIX
        shared[nm] = np.ascontiguousarray(v.reshape(NDC, 128).T)  # [128, NDC]

    def chan_major(x, dtype):
        # [B, PIX, C] -> [B, NCC, 128, PIX]
        xt = np.ascontiguousarray(x.reshape(B, PIX, CIN).transpose(0, 2, 1))
        return xt.reshape(B, NCC, 128, PIX).astype(dtype)

    def chan_major_padded(x):
        # [B, PIX, C] -> [B, NCC, 128, 30*30] bf16 with zero borders
        cm = chan_major(x, bf16).reshape(B, NCC, 128, H, W)
        out = np.zeros((B, NCC, 128, PAD, PAD), dtype=bf16)
        out[:, :, :, 1:29, 1:29] = cm
        return out.reshape(B, NCC, 128, XTLEN)

    xin = chan_major_padded(np.asarray(inputs["inputs"], dtype=f32))
    xhp = chan_major_padded(np.asarray(inputs["hidden_prev"], dtype=f32))
    # state in row-stride-30 layout (junk cols zero)
    spcm = chan_major(np.asarray(inputs["state_prev"], dtype=f32), f32)
    xsp = np.zeros((B, NCC, 128, H, PAD), f32)
    xsp[:, :, :, :, 0:W] = spcm.reshape(B, NCC, 128, H, W)
    xsp = xsp.reshape(B, NCC, 128, PPIX)

    in_maps = []
    for k in range(N_CORES):
        sl = slice(k * BL, (k + 1) * BL)
        m = dict(shared)
        m["inputs_t"] = xin[sl]
        m["hidden_t"] = xhp[sl]
        m["state_t"] = xsp[sl]
        m["gate_idx"] = _gate_idx(k)
        in_maps.append(m)
    return in_maps


def kernel(**inputs):
    nc = _get_nc()
    in_maps = _make_in_maps(inputs)
    res = run_bass_kernel_spmd(nc, in_maps, core_ids=list(range(N_CORES)))

    def unshard(name):
        # per-core outputs are [BL, NDC, 128, 28*30] channel-major with 2
        # junk columns per row; strip then restore NHWC
        full = np.concatenate([res.results[k][name] for k in range(N_CORES)],
                              axis=0)
        full = full.reshape(B, NDC, 128, H, PAD)[:, :, :, :, 0:W]
        full = full.reshape(B, D, PIX)
        return np.ascontiguousarray(full.transpose(0, 2, 1)).reshape(
            B, H, W, D)

    return unshard("hidden"), unshard("state")


# revision 50
# speedup vs baseline: 1.0387x; 1.0387x over previous
"""Trainium2 Bass kernel for nn_DeformableConvLSTMCell_33895881900284.

Full (unsharded) inputs in, full outputs out. Data-parallel over batch across
8 NeuronCores (8 batches per core), conv weights / gate params replicated.

Math per the reference:
  outI  = conv3x3_same(inputs, wconvInput)
  g     = tanh(outI + conv3x3_same(hidden_prev, wconvHidden) + gateBias)
  gapI  = mean_hw(outI);  gapH = mean_hw(hidden_prev)          # [B, D]
  i/f/o = sigmoid(wx*gapI + wh*gapH + bias)                    # [B, D]
  tiled gate: value used at (b, h, w, c) is gate[(28*b + h) % 64, c]
  state  = f*state_prev + i*g;  hidden = o*tanh(state)

v2 design (vs the f32r baseline):
  * Host pre-transposes inputs/hidden/state to channel-major [BL, cc, 128,
    784] (layout-only, like the baseline's host-side output untranspose), so
    the kernel does no PE transposes and no DVE layout copies.
  * The whole conv path runs in bf16: weights/inputs/gateBias are cast on
    the host, conv = 36 shifted bf16 matmuls + a bias identity-matmul per
    392-pixel window accumulating in fp32 PSUM. bf16 enables the PE's Fast
    Weight Load path (fp32r disables it), roughly halving per-matmul cost.
  * gapI never touches the conv output: by linearity 784*gapI is a
    combination of 9 masked pixel sums of the raw input with host-folded
    A-matrices (sums of conv taps). The masked sums (full/edge/corner) are
    free-dim DVE reductions over the staged channel-major tiles, so the
    AllGather of gate drivers fires ~45us in, long before the first
    elementwise consumer.
  * The (28*b+h)%64 gate scrambling makes gates cross-batch: cores AllGather
    their local GAP columns and a per-core index-array input drives an
    indirect-DMA gather of exactly the gate rows this core needs (the SPMD
    program stays identical across cores; only input data differs).
"""
import numpy as np
import ml_dtypes

import bass_rust
import concourse.bass as bass
import concourse.mybir as mybir
import concourse.tile as tile
from concourse.bass_utils import run_bass_kernel_spmd

F32 = mybir.dt.float32
BF16 = mybir.dt.bfloat16
I32 = mybir.dt.int32
AF = mybir.ActivationFunctionType
ALU = mybir.AluOpType

N_CORES = 8
B, H, W, CIN, D = 64, 28, 28, 256, 256
BL = B // N_CORES          # local batches per core
PIX = H * W                # 784
PAD = 30                   # padded row/col length
XTLEN = PAD * PAD          # 900
NW = 2                     # windows per batch
WROWS = H // NW            # 14
WN = WROWS * W             # 392
NCC = CIN // 128           # 2 input-channel chunks
NDC = D // 128             # 2 output-channel chunks

# tap order t = 3*kh + kw ; dh = kh-1, dw = kw-1
TAPS = [(kh, kw) for kh in range(3) for kw in range(3)]

# ---------------------------------------------------------------------------
# walrus fixup: split semaphore waits that exceed the per-instruction budget
MAX_WAITS = 1


def _split_excess_sem_waits(nc):
    counter = [0]
    for fn in nc.m.functions:
        for bb in fn.blocks:
            insts = bb.instructions
            i = 0
            while i < len(insts):
                inst = insts[i]
                si = inst.sync_info
                if si is not None and si.on_wait and len(si.on_wait) > MAX_WAITS:
                    waits = list(si.on_wait)
                    excess = waits[:-MAX_WAITS]
                    keep = waits[-MAX_WAITS:]
                    new_insts = []
                    for j in range(0, len(excess), MAX_WAITS):
                        chunk = excess[j:j + MAX_WAITS]
                        noop = mybir.InstNoOp(
                            name=f"I-waitsplit-{counter[0]}", ins=[], outs=[])
                        counter[0] += 1
                        noop.engine = inst.engine
                        noop.sync_info = bass_rust.SyncInfo(
                            on_wait=chunk, on_update=[])
                        nc.register_instruction(noop)
                        new_insts.append(noop)
                    inst.sync_info = bass_rust.SyncInfo(
                        on_wait=keep, on_update=list(si.on_update))
                    insts[i:i] = new_insts
                    i += len(new_insts)
                i += 1
    return nc


# ---------------------------------------------------------------------------
def _r3(ap, w):
    """view flat free dim as (rows, w)"""
    return ap.rearrange("c (r w) -> c r w", w=w)


def build_nc():
    nc = bass.Bass("TRN2", target_bir_lowering=False, debug=False,
                   num_devices=N_CORES)

    dram = {}
    dram["xin"] = nc.dram_tensor("inputs_t", [BL, NCC, 128, XTLEN], BF16,
                                 kind="ExternalInput").ap()
    dram["xhid"] = nc.dram_tensor("hidden_t", [BL, NCC, 128, XTLEN], BF16,
                                  kind="ExternalInput").ap()
    dram["xsp"] = nc.dram_tensor("state_t", [BL, NCC, 128, PIX], F32,
                                 kind="ExternalInput").ap()
    dram["w1"] = nc.dram_tensor("w1_bf", [9, NCC, 128, D], BF16,
                                kind="ExternalInput").ap()
    dram["w2"] = nc.dram_tensor("w2_bf", [9, NCC, 128, D], BF16,
                                kind="ExternalInput").ap()
    dram["afold"] = nc.dram_tensor("a_fold", [NCC, 9, 128, D], BF16,
                                   kind="ExternalInput").ap()
    dram["gb"] = nc.dram_tensor("gbias_t", [NDC, 128, PIX], F32,
                                kind="ExternalInput").ap()
    dram["vec"] = {}
    for nm in ("wxi", "whi", "inputBias", "wxf", "whf", "forgetBias",
               "wxo", "who", "outputBias"):
        dram["vec"][nm] = nc.dram_tensor(nm, [128, NDC], F32,
                                         kind="ExternalInput").ap()
    dram["ident"] = nc.dram_tensor("identity", [128, 128], F32,
                                   kind="ExternalInput").ap()
    dram["idx"] = nc.dram_tensor("gate_idx", [4, 224, 1], I32,
                                 kind="ExternalInput").ap()
    dram["hidden"] = nc.dram_tensor("hidden", [BL, NDC, 128, PIX], F32,
                                    kind="ExternalOutput").ap()
    dram["state"] = nc.dram_tensor("state", [BL, NDC, 128, PIX], F32,
                                   kind="ExternalOutput").ap()
    dram["cc_in"] = nc.dram_tensor("cc_in", [32, 128], F32, kind="Internal").ap()
    dram["cc_out"] = nc.dram_tensor("cc_out", [N_CORES * 32, 128], F32,
                                    kind="Internal", addr_space="Shared").ap()

    ctx_mgr = nc.allow_low_precision("bf16 conv path")
    ctx_mgr.__enter__()
    with tile.TileContext(nc) as tc:
        _build_body(nc, tc, dram)
    ctx_mgr.__exit__(None, None, None)
    return nc


def _build_body(nc, tc, dram):
    from contextlib import ExitStack
    ctx = ExitStack()
    pool = lambda **kw: ctx.enter_context(tc.tile_pool(**kw))

    const = pool(name="const", bufs=1)
    xspp = pool(name="xspp", bufs=6)       # [128, 784] f32 state tiles
    gtp = pool(name="gtp", bufs=16)        # [128, 392] f32 tanh-conv tiles
    outb = pool(name="outb", bufs=4)       # [128, 784] f32 out tiles (per tag)
    ew = pool(name="ew", bufs=3)           # [128, 392] f32 scratch (per tag)
    gtmp = pool(name="gtmp", bufs=2)
    gath = pool(name="gath", bufs=2)
    ps_conv = pool(name="ps_conv", bufs=6, space="PSUM")
    ps_gap = pool(name="ps_gap", bufs=1, space="PSUM")
    ps_tr = pool(name="ps_tr", bufs=1, space="PSUM")

    # ---- constant tiles; DMA emission deferred to the schedule section so
    # each ring's order matches need order ----
    wc = [const.tile([128, 9 * NCC * D], BF16, tag=f"wc{conv}",
                     name=f"wc{conv}") for conv in range(2)]

    def wblk(conv, t, cc):
        off = (t * NCC + cc) * D
        return wc[conv][:, off:off + D]

    def emit_wtap(conv, t, eng):
        # one tap's [NCC,128,D] chunk -> wc[conv][:, t*NCC*D : (t+1)*NCC*D]
        dst = wc[conv][:, t * NCC * D:(t + 1) * NCC * D]
        eng.dma_start(
            dst.rearrange("p (c n) -> p c n", c=NCC),
            dram["w1" if conv == 0 else "w2"][t].rearrange("c p n -> p c n"))

    gbias = [const.tile([128, PIX], F32, tag=f"gbias{dc}", name=f"gbias{dc}")
             for dc in range(NDC)]

    afold = const.tile([128, NCC * 9 * D], BF16, tag="afold")

    def ablk(cc, g):
        off = (cc * 9 + g) * D
        return afold[:, off:off + D]

    idx_sb = []
    for g4 in range(4):
        idx_sb.append([const.tile([112, 1], I32, tag=f"idx{g4}_{hf}",
                                  name=f"idx{g4}_{hf}")
                       for hf in range(2)])

    vecs = {nm: const.tile([128, NDC], F32, tag=f"vec_{nm}",
                           name=f"vec_{nm}")
            for nm in dram["vec"]}

    ident = const.tile([128, 128], F32, tag="ident")

    def emit_late_consts():
        # scalar ring, after the hidden loads: needed from ~35us on
        for dc in range(NDC):
            nc.scalar.dma_start(gbias[dc][:], dram["gb"][dc])
        nc.scalar.dma_start(
            afold[:].rearrange("p (c g n) -> p c g n", c=NCC, g=9),
            dram["afold"][:].rearrange("c g p n -> p c g n"))
        nc.scalar.dma_start(ident[:], dram["ident"][:])
        for g4 in range(4):
            for hf in range(2):
                nc.scalar.dma_start(idx_sb[g4][hf][:],
                                    dram["idx"][g4, hf * 112:(hf + 1) * 112, :])
        for nm in dram["vec"]:
            nc.scalar.dma_start(vecs[nm][:], dram["vec"][nm][:])

    # ---- padded input tiles (host pre-pads the zero borders) ----
    xt = {}       # xt[(j, tensor, cc)] -> [128, 900] bf16
    for j in range(BL):
        for tn in ("in", "hid"):
            for cc in range(NCC):
                xt[(j, tn, cc)] = const.tile(
                    [128, XTLEN], BF16, tag=f"x{tn}{j}_{cc}",
                    name=f"x{tn}{j}_{cc}")

    # masked-sum accumulators (columns written per batch)
    rawI = [const.tile([128, 9 * BL], BF16, tag=f"rawI{cc}", name=f"rawI{cc}")
            for cc in range(NCC)]
    rawH = [const.tile([128, BL], F32, tag=f"rawH{cc}", name=f"rawH{cc}")
            for cc in range(NCC)]

    # ---- per-batch input loading: xin on the sync ring, xhid on the
    # scalar ring (parallel rings -> reductions finish ~32us in) ----
    def emit_load_xin(j):
        for cc in range(NCC):
            nc.sync.dma_start(xt[(j, "in", cc)][:], dram["xin"][j, cc])

    def emit_load_xhid(j):
        for cc in range(NCC):
            nc.scalar.dma_start(xt[(j, "hid", cc)][:], dram["xhid"][j, cc])

    xsp_tiles = {}

    def emit_load_state(j):
        ts = []
        for cc in range(NCC):
            s = xspp.tile([128, PIX], F32, tag="xsp", name=f"xsp{j}_{cc}")
            nc.sync.dma_start(s[:], dram["xsp"][j, cc])
            ts.append(s)
        xsp_tiles[j] = ts

    # ---- stage: masked-sum reductions (vector) over the padded tiles ----
    def emit_stage(j):
        for tn in ("in", "hid"):
            for cc in range(NCC):
                s = xt[(j, tn, cc)]
                s3 = _r3(s[:], PAD)                     # [128, 30, 30]
                if tn == "hid":
                    # gapH: full pixel sum only (borders are zero)
                    nc.vector.tensor_reduce(
                        rawH[cc][:, j:j + 1], s[:], mybir.AxisListType.X,
                        ALU.add)
                else:
                    rv = rawI[cc][:].rearrange("c (g b) -> c g b", b=BL)
                    # group order: S, Rf(row0), Rl(row27), Cf(col0),
                    # Cl(col27), K00, K0L, KL0, KLL
                    nc.vector.tensor_reduce(
                        rv[:, 0, j:j + 1], s[:], mybir.AxisListType.X, ALU.add)
                    nc.vector.tensor_reduce(
                        rv[:, 1, j:j + 1], s[:, PAD + 1:PAD + 1 + W],
                        mybir.AxisListType.X, ALU.add)
                    nc.vector.tensor_reduce(
                        rv[:, 2, j:j + 1], s[:, 28 * PAD + 1:28 * PAD + 1 + W],
                        mybir.AxisListType.X, ALU.add)
                    nc.vector.tensor_reduce(
                        rv[:, 3, j:j + 1], s3[:, 1:29, 1:2],
                        mybir.AxisListType.XY, ALU.add)
                    nc.vector.tensor_reduce(
                        rv[:, 4, j:j + 1], s3[:, 1:29, 28:29],
                        mybir.AxisListType.XY, ALU.add)
                    corners = s3[:, 1:29:27, 1:29:27]   # [128, 2, 2]
                    dstc = rv[:, 5:9, j:j + 1].rearrange(
                        "c (x y) o -> c x (y o)", x=2)
                    nc.vector.tensor_copy(dstc, corners)

    # ---- conv windows ----
    gt_tiles = {}

    def emit_conv_w(j, wi, dc):
        h0 = 1 + wi * WROWS
        base = (h0 - 1) * W
        p = ps_conv.tile([128, WN], F32, tag="pconv", name="pconv")
        p3 = _r3(p[:], W)
        first = True
        for conv, tn in ((0, "in"), (1, "hid")):
            for t, (kh, kw) in enumerate(TAPS):
                dh, dwid = kh - 1, kw - 1
                for cc in range(NCC):
                    rhs = _r3(xt[(j, tn, cc)][:], PAD)[
                        :, h0 + dh:h0 + dh + WROWS,
                        1 + dwid:1 + dwid + W]
                    last = (conv == 1 and t == 8 and cc == NCC - 1)
                    nc.tensor.matmul(
                        p3, wblk(conv, t, cc)[:, dc * 128:(dc + 1) * 128],
                        rhs, start=first, stop=last)
                    first = False
        nc.vector.tensor_tensor(out=p[:], in0=p[:],
                                in1=gbias[dc][:, base:base + WN],
                                op=ALU.add)
        gt = gtp.tile([128, WN], F32, tag="gt", name="gt")
        nc.scalar.activation(gt[:], p[:], AF.Tanh)
        gt_tiles[(j, wi, dc)] = gt

    def emit_conv(j):
        for wi in range(NW):
            for dc in range(NDC):
                emit_conv_w(j, wi, dc)

    # ---- gap combine + AllGather ----
    def emit_combine():
        gap_ps = ps_gap.tile([8, D], F32, tag="gapI")
        for cc in range(NCC):
            rv = rawI[cc][:].rearrange("c (g b) -> c g b", b=BL)
            for g in range(9):
                nc.tensor.matmul(gap_ps[:], rv[:, g], ablk(cc, g),
                                 start=(cc == 0 and g == 0),
                                 stop=(cc == NCC - 1 and g == 8))
        gapI_sb = const.tile([8, D], F32, tag="gapI_sb")
        nc.vector.tensor_copy(gapI_sb[:], gap_ps[:])
        nc.scalar.dma_start(dram["cc_in"][0:8, :], gapI_sb[:, 0:128])
        nc.scalar.dma_start(dram["cc_in"][8:16, :], gapI_sb[:, 128:256])
        for cc in range(NCC):
            pt = ps_tr.tile([128, 128], F32, tag="ptr", name="pt_gapH")
            pt = pt[0:8, :]
            nc.tensor.transpose(pt, rawH[cc][:], ident[:])
            hs = const.tile([8, 128], F32, tag=f"gapH_sb{cc}",
                            name=f"gapHsb{cc}")
            nc.vector.tensor_copy(hs[:], pt)
            nc.scalar.dma_start(dram["cc_in"][16 + 8 * cc:24 + 8 * cc, :],
                                hs[:])
        nc.gpsimd.collective_compute(
            "AllGather", ALU.bypass, replica_groups=[list(range(N_CORES))],
            ins=[dram["cc_in"][:]], outs=[dram["cc_out"][:]])

    # ---- gather + gate tables ----
    gates = {}

    def emit_gates():
        sel = [const.tile([128, 224], F32, tag=f"sel{g4}", name=f"sel{g4}")
               for g4 in range(4)]
        for g4 in range(4):
            for hf in range(2):
                gtile = gath.tile([112, 128], F32, tag="gath", name="gath")
                nc.gpsimd.indirect_dma_start(
                    out=gtile[:], out_offset=None, in_=dram["cc_out"][:],
                    in_offset=bass.IndirectOffsetOnAxis(
                        ap=idx_sb[g4][hf][:, :1], axis=0))
                pt = ps_tr.tile([128, 128], F32, tag="ptr", name="pt_gath")
                nc.tensor.transpose(pt[:, 0:112], gtile[:],
                                    ident[0:112, 0:112])
                nc.vector.tensor_copy(sel[g4][:, hf * 112:(hf + 1) * 112],
                                      pt[:, 0:112])
        for gate, wx, wh, bi in (("i", "wxi", "whi", "inputBias"),
                                 ("f", "wxf", "whf", "forgetBias"),
                                 ("o", "wxo", "who", "outputBias")):
            per_dc = []
            for dc in range(NDC):
                t1 = gtmp.tile([128, 224], F32, tag="gm1", name="gm1")
                nc.vector.tensor_scalar_mul(t1[:], sel[dc][:],
                                            vecs[wx][:, dc:dc + 1])
                t2 = gtmp.tile([128, 224], F32, tag="gm2", name="gm2")
                nc.vector.tensor_scalar_mul(t2[:], sel[2 + dc][:],
                                            vecs[wh][:, dc:dc + 1])
                nc.vector.tensor_tensor(out=t1[:], in0=t1[:], in1=t2[:],
                                        op=ALU.add)
                gt = const.tile([128, 224], F32, tag=f"gate_{gate}{dc}",
                                name=f"gate_{gate}{dc}")
                nc.scalar.activation(gt[:], t1[:], AF.Sigmoid,
                                     bias=vecs[bi][:, dc:dc + 1])
                per_dc.append(gt)
            gates[gate] = per_dc

    # ---- elementwise + store (per-window so the tail drains fast) ----
    out_tiles = {}

    def emit_ew_w(j, wi, dc):
        if j not in out_tiles:
            out_tiles[j] = (
                [outb.tile([128, PIX], F32, tag="stT", name=f"stT{j}_{dc}")
                 for dc in range(NDC)],
                [outb.tile([128, PIX], F32, tag="hidT", name=f"hidT{j}_{dc}")
                 for dc in range(NDC)])
        stT, hidT = out_tiles[j]
        h0 = 1 + wi * WROWS
        base = (h0 - 1) * W
        t0 = j * H + (h0 - 1)

        def gw(gate):
            return gates[gate][dc][:, t0:t0 + WROWS].to_broadcast(
                [128, WROWS, W])

        gt = gt_tiles.pop((j, wi, dc))
        sp3 = _r3(xsp_tiles[j][dc][:, base:base + WN], W)
        g3 = _r3(gt[:], W)
        st3 = _r3(stT[dc][:, base:base + WN], W)
        hd3 = _r3(hidT[dc][:, base:base + WN], W)
        s1 = ew.tile([128, WN], F32, tag="s1", name="s1")
        nc.gpsimd.tensor_tensor(out=_r3(s1[:], W), in0=sp3,
                                in1=gw("f"), op=ALU.mult)
        s2 = ew.tile([128, WN], F32, tag="s2", name="s2")
        nc.gpsimd.tensor_tensor(out=_r3(s2[:], W), in0=g3,
                                in1=gw("i"), op=ALU.mult)
        nc.vector.tensor_tensor(out=st3, in0=_r3(s1[:], W),
                                in1=_r3(s2[:], W), op=ALU.add)
        th = ew.tile([128, WN], F32, tag="th", name="th")
        nc.scalar.activation(th[:], stT[dc][:, base:base + WN],
                             AF.Tanh)
        nc.gpsimd.tensor_tensor(out=hd3, in0=_r3(th[:], W),
                                in1=gw("o"), op=ALU.mult)

    def emit_store(j):
        stT, hidT = out_tiles.pop(j)
        for dname, buf in (("state", stT), ("hidden", hidT)):
            for dc in range(NDC):
                nc.scalar.dma_start(dram[dname][j, dc], buf[dc][:])

    # ================= schedule =================
    # sync ring: xin(0), w1 taps, xin(1..7), state(0..7)
    # scalar ring: xhid(0), w2 taps, xhid(1..7), gbias/afold/ident/idx/vec,
    #              cc_in, stores
    emit_load_xin(0)
    for t in range(9):
        emit_wtap(0, t, nc.sync)
    emit_load_xhid(0)
    for t in range(9):
        emit_wtap(1, t, nc.scalar)
    for j in range(1, BL):
        emit_load_xin(j)
        emit_load_xhid(j)
    emit_late_consts()
    for j in range(BL):
        emit_load_state(j)
    for j in range(BL):
        emit_stage(j)
    emit_conv(0)
    emit_combine()
    emit_conv(1)
    emit_conv(2)
    emit_conv_w(3, 0, 0)
    emit_conv_w(3, 0, 1)
    emit_conv_w(3, 1, 0)
    emit_gates()
    emit_conv_w(3, 1, 1)
    for wi in range(NW):
        for dc in range(NDC):
            emit_ew_w(0, wi, dc)
    emit_store(0)
    for j in range(4, BL):
        for wi in range(NW):
            for dc in range(NDC):
                emit_conv_w(j, wi, dc)
                emit_ew_w(j - 3, wi, dc)
        emit_store(j - 3)
    for j in range(BL - 3, BL):
        for wi in range(NW):
            for dc in range(NDC):
                emit_ew_w(j, wi, dc)
        emit_store(j)

    ctx.close()


# ---------------------------------------------------------------------------
_NC_CACHE = None


def _get_nc():
    global _NC_CACHE
    if _NC_CACHE is None:
        nc = build_nc()
        _split_excess_sem_waits(nc)
        _NC_CACHE = nc
    return _NC_CACHE


def _gate_idx(core):
    idx = np.empty((4, 224, 1), np.int32)
    for j in range(BL):
        for hh in range(H):
            t = j * H + hh
            sel_b = (H * (BL * core + j) + hh) % B
            cp, bp = sel_b // BL, sel_b % BL
            for g in range(4):
                idx[g, t, 0] = cp * 32 + g * 8 + bp
    return idx


def _make_in_maps(inputs):
    f32 = np.float32
    bf16 = ml_dtypes.bfloat16

    w1 = np.ascontiguousarray(inputs["wconvInput"], dtype=f32)  # [3,3,CIN,D]
    w2 = np.ascontiguousarray(inputs["wconvHidden"], dtype=f32)
    # w{1,2}_bf[t, cc, 128, D]
    w1b = np.empty((9, NCC, 128, D), dtype=bf16)
    w2b = np.empty((9, NCC, 128, D), dtype=bf16)
    for wb, w in ((w1b, w1), (w2b, w2)):
        for t, (kh, kw) in enumerate(TAPS):
            for cc in range(NCC):
                wb[t, cc] = w[kh, kw, cc * 128:(cc + 1) * 128, :]

    # A-fold for gapI: 784*gapI = sum_g raw_g^T @ A_g  (group order
    # S, Rf, Rl, Cf, Cl, K00, K0L, KL0, KLL; edge groups negated)
    wt = w1.reshape(9, CIN, D)
    A = np.empty((9, CIN, D), f32)
    A[0] = wt.sum(0)
    A[1] = -(wt[6] + wt[7] + wt[8])
    A[2] = -(wt[0] + wt[1] + wt[2])
    A[3] = -(wt[2] + wt[5] + wt[8])
    A[4] = -(wt[0] + wt[3] + wt[6])
    A[5], A[6], A[7], A[8] = wt[8], wt[6], wt[2], wt[0]
    afold = np.empty((NCC, 9, 128, D), dtype=bf16)
    for cc in range(NCC):
        afold[cc] = A[:, cc * 128:(cc + 1) * 128, :]

    gb = np.ascontiguousarray(inputs["gateBias"], dtype=f32).reshape(PIX, D)
    gbias_t = np.ascontiguousarray(gb.T.reshape(NDC, 128, PIX))

    shared = {
        "w1_bf": w1b,
        "w2_bf": w2b,
        "a_fold": afold,
        "gbias_t": gbias_t,
        "identity": np.eye(128, dtype=f32),
    }
    for nm in ("wxi", "whi", "inputBias", "wxf", "whf", "forgetBias",
               "wxo", "who", "outputBias"):
        v = np.ascontiguousarray(inputs[nm], dtype=f32).reshape(D)
        if nm.startswith("wx") or nm.startswith("wh"):
            v = v / PIX
        shared[nm] = np.ascontiguousarray(v.reshape(NDC, 128).T)  # [128, NDC]

    def chan_major(x, dtype):
        # [B, PIX, C] -> [B, NCC, 128, PIX]
        xt = np.ascontiguousarray(x.reshape(B, PIX, CIN).transpose(0, 2, 1))
        return xt.reshape(B, NCC, 128, PIX).astype(dtype)

    def chan_major_padded(x):
        # [B, PIX, C] -> [B, NCC, 128, 30*30] bf16 with zero borders
        cm = chan_major(x, bf16).reshape(B, NCC, 128, H, W)
        out = np.zeros((B, NCC, 128, PAD, PAD), dtype=bf16)
        out[:, :, :, 1:29, 1:29] = cm
        return out.reshape(B, NCC, 128, XTLEN)

    xin = chan_major_padded(np.asarray(inputs["inputs"], dtype=f32))
    xhp = chan_major_padded(np.asarray(inputs["hidden_prev"], dtype=f32))
    xsp = chan_major(np.asarray(inputs["state_prev"], dtype=f32), f32)

    in_maps = []
    for k in range(N_CORES):
        sl = slice(k * BL, (k + 1) * BL)
        m = dict(shared)
        m["inputs_t"] = xin[sl]
        m["hidden_t"] = xhp[sl]
        m["state_t"] = xsp[sl]
        m["gate_idx"] = _gate_idx(k)
        in_maps.append(m)
    return in_maps


def kernel(**inputs):
    nc = _get_nc()
    in_maps = _make_in_maps(inputs)
    res = run_bass_kernel_spmd(nc, in_maps, core_ids=list(range(N_CORES)))

    def unshard(name):
        # per-core outputs are [BL, NDC, 128, PIX] (channel-major)
        full = np.concatenate([res.results[k][name] for k in range(N_CORES)],
                              axis=0)
        return np.ascontiguousarray(full.transpose(0, 3, 1, 2)).reshape(
            B, H, W, D)

    return unshard("hidden"), unshard("state")
